# revision 8
# baseline (speedup 1.0000x reference)
"""Trainium2 Bass kernel for nn_ConvModule: LN -> 1x1 conv (D->2I) -> SwiGLU
-> depthwise conv (K=31) -> PReLU -> 1x1 conv (I->D).

Sharding: data-parallel over batch, 2 batches per core across 8 cores.

v4 design (from v3):
  - LN stats via DVE bn_stats/bn_aggr (one pass) instead of two ACT accum
    passes; normalize stays on ACT (per-partition scale/bias APs).
  - SwiGLU combine moved off DVE: ACT copies ps_a(+b1a) -> a_sb bf16, then
    GPSIMD tensor_tensor multiply a_sb*s_sb -> strip.
  - xn^T PSUM->SBUF copies moved to ACT.
  - Depthwise split 9/7: PE diag-matmul units b0:{0..3} b1:{0..4} (only 5
    diag sets needed), DVE chain units b0:{4..7} b1:{5..7}.
  - b0 DVE chains full-width (2048) to halve instruction overheads; b1
    chains in halves so GEMM2 panels can start early.
  - b2 always via K=1 ones-row matmul; GEMM2 PSUM->SBUF copies on ACT.
"""

import sys

sys.path.insert(0, "/opt/trn_rl_repo")

from contextlib import ExitStack

import numpy as np
import ml_dtypes

import concourse.bacc as bacc
import concourse.tile as tile
from concourse import mybir
from concourse.masks import make_identity
from concourse.bass_utils import run_bass_kernel_spmd

B, T, D, I, K = 16, 2048, 512, 1024, 31
NCORES = 8
BPC = B // NCORES  # batches per core
E = 2 * I  # 2048
TP = T // 512  # time panels per batch (4)
CB = I // 128  # channel blocks (8)
DCH = D // 128  # d chunks (4)
PADL = 16  # left pad (even so DVE chain windows stay 4B-aligned)
STRIPW = PADL + T + 16  # 2080
P = 128

F32 = mybir.dt.float32
BF16 = mybir.dt.bfloat16
ALU = mybir.AluOpType
ACTF = mybir.ActivationFunctionType

# depthwise unit split (per batch): cbs on the PE vs on DVE chains
PE_CBS_BY_B = {0: (0, 1, 2, 3), 1: (0, 1, 2, 3, 4)}
DVE_CBS_BY_B = {0: (4, 5, 6, 7), 1: (5, 6, 7)}
DIAG_CBS = (0, 1, 2, 3, 4)  # union of PE cbs; host preps these diag sets
DIAG_IDX = {cb: i for i, cb in enumerate(DIAG_CBS)}


def _build_kernel(ctx, tc):
    nc = tc.nc
    x_d = nc.dram_tensor("x", [BPC, T, D], F32, kind="ExternalInput").ap()
    w1p_d = nc.dram_tensor("w1p", [D, E], BF16, kind="ExternalInput").ap()
    b1p_d = nc.dram_tensor("b1p", [E], F32, kind="ExternalInput").ap()
    w2t_d = nc.dram_tensor("w2t", [I, D], BF16, kind="ExternalInput").ap()
    diag_d = nc.dram_tensor(
        "diag", [P, len(DIAG_CBS) * K * P], BF16, kind="ExternalInput").ap()
    dwc_d = nc.dram_tensor("dwc", [P, CB * K], F32, kind="ExternalInput").ap()
    dwb_d = nc.dram_tensor("dwbc", [P, CB], F32, kind="ExternalInput").ap()
    alpha_d = nc.dram_tensor("alphac", [P, CB], F32, kind="ExternalInput").ap()
    b2r_d = nc.dram_tensor("b2r", [1, D], BF16, kind="ExternalInput").ap()
    ones_d = nc.dram_tensor("ones", [1, P], BF16, kind="ExternalInput").ap()
    out_d = nc.dram_tensor("out", [BPC, T, D], F32, kind="ExternalOutput").ap()

    const = ctx.enter_context(tc.tile_pool(name="const", bufs=1))
    psum = ctx.enter_context(tc.tile_pool(name="psum", bufs=6, space="PSUM"))

    ident_bf = const.tile([P, P], BF16, tag="ident_bf")
    make_identity(nc, ident_bf[:])

    # ---- constants / weights (all host-prepared) ----
    w1t = [const.tile([P, E], BF16, tag=f"w1t{j}", name=f"w1t{j}")
           for j in range(DCH)]
    w2t = [const.tile([P, D], BF16, tag=f"w2t{i}", name=f"w2t{i}")
           for i in range(CB)]
    b1p = const.tile([P, 2 * CB], F32, tag="b1p")
    dw_sb = const.tile([P, CB * K], F32, tag="dw_sb")
    dwb_sb = const.tile([P, CB], F32, tag="dwb_sb")
    alpha_sb = const.tile([P, CB], F32, tag="alpha_sb")
    b2row = const.tile([1, D], BF16, tag="b2row")
    ones_bf = const.tile([1, P], BF16, tag="ones_bf")
    eps_t = const.tile([P, 1], F32, tag="eps_t")
    nc.vector.memset(eps_t[:], 1e-5)
    diag_sb = const.tile([P, len(DIAG_CBS) * K * P], BF16, tag="diag")

    def load_mid_consts():
        for j in range(DCH):
            nc.sync.dma_start(w1t[j][:], w1p_d[j * P:(j + 1) * P, :])
        nc.sync.dma_start(b1p[:], b1p_d.rearrange("(i p) -> p i", p=P))
        nc.sync.dma_start(dw_sb[:], dwc_d)
        nc.sync.dma_start(dwb_sb[:], dwb_d)
        nc.sync.dma_start(alpha_sb[:], alpha_d)
        nc.sync.dma_start(b2row[:], b2r_d)
        nc.sync.dma_start(ones_bf[:], ones_d)
        nc.sync.dma_start(diag_sb[:], diag_d)

    def load_late_consts():
        for i in range(CB):
            nc.sync.dma_start(w2t[i][:], w2t_d[i * P:(i + 1) * P, :])

    # ---- pools ----
    xpool = ctx.enter_context(tc.tile_pool(name="xpool", bufs=2))
    stat = ctx.enter_context(tc.tile_pool(name="stat", bufs=4))
    xnp = ctx.enter_context(tc.tile_pool(name="xnp", bufs=4))
    xntp = ctx.enter_context(tc.tile_pool(name="xntp", bufs=6))
    sw = ctx.enter_context(tc.tile_pool(name="sw", bufs=4))
    strips = ctx.enter_context(tc.tile_pool(name="strips", bufs=8))
    soddp = ctx.enter_context(tc.tile_pool(name="soddp", bufs=4))
    accp = ctx.enter_context(tc.tile_pool(name="accp", bufs=4))
    prodp = ctx.enter_context(tc.tile_pool(name="prodp", bufs=2))
    vtp = ctx.enter_context(tc.tile_pool(name="vtp", bufs=8))
    outp = ctx.enter_context(tc.tile_pool(name="outp", bufs=2))

    strip = {}
    sodd = {}
    vt = {}

    def load_x_panel(b, tp):
        tiles = []
        for tt in range(4):
            t0 = tp * 512 + tt * P
            x_t = xpool.tile([P, D], F32, tag="x", bufs=7,
                             name=f"x_{b}_{tp}_{tt}")
            nc.sync.dma_start(x_t[:], x_d[b, t0:t0 + P, :])
            tiles.append(x_t)
        return tiles

    xq = {}

    def emit_A_panel(b, tp):
        """LN stats + normalize + transpose + GEMM1 + SwiGLU for one panel."""
        if tp + 1 < TP:
            if (b, tp + 1) not in xq:
                xq[(b, tp + 1)] = load_x_panel(b, tp + 1)
        elif b + 1 < BPC:
            xq[(b + 1, 0)] = load_x_panel(b + 1, 0)
        x_tiles = xq.pop((b, tp))

        # one-pass LN stats on DVE: bn_stats -> bn_aggr (mean, var per token)
        mv6 = stat.tile([P, 24], F32, tag="mv6")
        agg = stat.tile([P, 8], F32, tag="agg")
        for tt in range(4):
            nc.vector.bn_stats(mv6[:, 6 * tt:6 * tt + 6], x_tiles[tt][:])
            nc.vector.bn_aggr(agg[:, 2 * tt:2 * tt + 2],
                              mv6[:, 6 * tt:6 * tt + 6])
        agg3 = agg[:].rearrange("p (i two) -> p i two", two=2)
        mean4 = agg3[:, :, 0]
        var4 = agg3[:, :, 1]
        rstd4 = stat.tile([P, 4], F32, tag="rstd4")
        nc.scalar.activation(rstd4[:], var4, ACTF.Abs_reciprocal_sqrt,
                             bias=eps_t[:])
        negm4 = stat.tile([P, 4], F32, tag="negm4")
        nc.vector.tensor_scalar_mul(negm4[:], mean4, -1.0)
        negmr4 = stat.tile([P, 4], F32, tag="negmr4")
        nc.vector.tensor_mul(negmr4[:], negm4[:], rstd4[:])

        xn_tiles = []
        for tt in range(4):
            xn_t = xnp.tile([P, D], BF16, tag="xn")
            nc.scalar.activation(xn_t[:], x_tiles[tt][:], ACTF.Identity,
                                 bias=negmr4[:, tt:tt + 1],
                                 scale=rstd4[:, tt:tt + 1])
            xn_tiles.append(xn_t)
        xnt_p = []
        for j in range(DCH):
            ptr = psum.tile([P, 512], BF16, tag="pst", bufs=2)
            for tt in range(4):
                nc.tensor.transpose(ptr[:, tt * P:(tt + 1) * P],
                                    xn_tiles[tt][:, j * P:(j + 1) * P],
                                    ident_bf[:])
            xt = xntp.tile([P, 512], BF16, tag="xnt", name=f"xnt_{b}_{tp}_{j}")
            nc.scalar.activation(xt[:], ptr[:], ACTF.Copy)
            xnt_p.append(xt)

        for i in range(CB):
            ps_a = psum.tile([P, 512], F32, tag="ps")
            ps_g = psum.tile([P, 512], F32, tag="ps")
            for j in range(DCH):
                nc.tensor.matmul(
                    ps_a[:], w1t[j][:, i * P:(i + 1) * P], xnt_p[j][:],
                    start=(j == 0), stop=(j == DCH - 1))
            for j in range(DCH):
                nc.tensor.matmul(
                    ps_g[:], w1t[j][:, I + i * P:I + (i + 1) * P], xnt_p[j][:],
                    start=(j == 0), stop=(j == DCH - 1))
            s_sb = sw.tile([P, 512], BF16, tag="s_sb")
            nc.scalar.activation(s_sb[:], ps_g[:], ACTF.Silu,
                                 bias=b1p[:, CB + i:CB + i + 1])
            a_sb = sw.tile([P, 512], BF16, tag="a_sb")
            nc.scalar.activation(a_sb[:], ps_a[:], ACTF.Identity,
                                 bias=b1p[:, i:i + 1])
            nc.gpsimd.tensor_mul(
                strip[(b, i)][:, PADL + tp * 512:PADL + (tp + 1) * 512],
                a_sb[:], s_sb[:])

    def alloc_strips(b):
        for cb in range(CB):
            s = strips.tile([P, STRIPW], BF16, tag="strip",
                            name=f"strip_{b}_{cb}")
            nc.gpsimd.memset(s[:, 0:PADL], 0.0)
            nc.gpsimd.memset(s[:, PADL + T:STRIPW], 0.0)
            strip[(b, cb)] = s
            vt[(b, cb)] = vtp.tile([P, T], BF16, tag="vt",
                                   name=f"vt_{b}_{cb}")

    def emit_odd_copy(b, cb):
        so = soddp.tile([P, STRIPW], BF16, tag="sodd", name=f"sodd_{b}_{cb}")
        nc.vector.tensor_copy(so[:, 0:STRIPW - 2],
                              strip[(b, cb)][:, 1:STRIPW - 1])
        sodd[(b, cb)] = so

    def emit_conv_pe_cb(b, cb):
        """All 4 tp panels of one cb, tap-outer so each diagonal is loaded
        into the PE once (4 PSUM banks accumulate in parallel)."""
        ci = DIAG_IDX[cb]
        st = strip[(b, cb)]
        pcs = [psum.tile([P, 512], F32, tag="ps", name=f"psc_{b}_{cb}_{tp}")
               for tp in range(TP)]
        for k in range(K):
            dg = diag_sb[:, (ci * K + k) * P:(ci * K + k + 1) * P]
            for tp in range(TP):
                nc.tensor.matmul(
                    pcs[tp][:], dg,
                    st[:, tp * 512 + k + 1:tp * 512 + k + 1 + 512],
                    start=(k == 0), stop=(k == K - 1))
        for tp in range(TP):
            nc.scalar.activation(vt[(b, cb)][:, tp * 512:(tp + 1) * 512],
                                 pcs[tp][:], ACTF.Prelu,
                                 bias=dwb_sb[:, cb:cb + 1],
                                 alpha=alpha_sb[:, cb:cb + 1])

    def win(b, cb, t0, k, L):
        off = t0 + k + 1
        if off % 2 == 0:
            return strip[(b, cb)][:, off:off + L]
        return sodd[(b, cb)][:, off - 1:off - 1 + L]

    def emit_chain(b, cb, t0, L, acc=None):
        """product+add tree for strip cols [t0,t0+L), all taps, on DVE."""
        if acc is None:
            acc = accp.tile([P, T], BF16, tag="acc", name=f"acc_{b}_{cb}")
        wcol = lambda k: dw_sb[:, cb * K + k:cb * K + k + 1]
        a = acc[:, t0:t0 + L]
        nc.vector.tensor_scalar(a, win(b, cb, t0, 0, L), wcol(0),
                                dwb_sb[:, cb:cb + 1],
                                op0=ALU.mult, op1=ALU.add)
        for k in range(1, K):
            pk = prodp.tile([P, T], BF16, tag="pk", bufs=2)
            nc.vector.tensor_scalar_mul(pk[:, 0:L], win(b, cb, t0, k, L),
                                        wcol(k))
            nc.vector.tensor_add(a, a, pk[:, 0:L])
        return acc

    def emit_chain_prelu(b, cb, t0, L, acc):
        nc.scalar.activation(vt[(b, cb)][:, t0:t0 + L], acc[:, t0:t0 + L],
                             ACTF.Prelu, alpha=alpha_sb[:, cb:cb + 1])

    def emit_C(b, tp):
        for tt in range(4):
            ps_o = psum.tile([P, D], F32, tag="ps")
            c0 = tp * 512 + tt * P
            nc.tensor.matmul(ps_o[:], ones_bf[:], b2row[:],
                             start=True, stop=False)
            for cb in range(CB):
                nc.tensor.matmul(
                    ps_o[:], vt[(b, cb)][:, c0:c0 + P], w2t[cb][:],
                    start=False, stop=(cb == CB - 1))
            o_sb = outp.tile([P, D], F32, tag="o_sb")
            nc.scalar.activation(o_sb[:], ps_o[:], ACTF.Copy)
            nc.sync.dma_start(out_d[b, c0:c0 + P, :], o_sb[:])

    # ================= emission =================
    xq[(0, 0)] = load_x_panel(0, 0)
    alloc_strips(0)
    xq[(0, 1)] = load_x_panel(0, 1)
    load_mid_consts()
    emit_A_panel(0, 0)
    load_late_consts()
    for tp in range(1, TP):
        emit_A_panel(0, tp)
    for cb in DVE_CBS_BY_B[0]:
        emit_odd_copy(0, cb)

    # conv b0 PE part
    for cb in PE_CBS_BY_B[0]:
        emit_conv_pe_cb(0, cb)

    # b0 chains full-width, interleaved with b1 A panels
    acc0 = {}
    acc0[4] = emit_chain(0, 4, 0, T)
    alloc_strips(1)
    emit_A_panel(1, 0)
    acc0[5] = emit_chain(0, 5, 0, T)
    emit_A_panel(1, 1)
    acc0[6] = emit_chain(0, 6, 0, T)
    emit_A_panel(1, 2)
    acc0[7] = emit_chain(0, 7, 0, T)
    emit_A_panel(1, 3)
    for cb in DVE_CBS_BY_B[1]:
        emit_odd_copy(1, cb)
    for cb in DVE_CBS_BY_B[0]:
        emit_chain_prelu(0, cb, 0, T, acc0[cb])
    emit_C(0, 0)
    emit_C(0, 1)
    emit_C(0, 2)
    emit_C(0, 3)

    # conv b1 PE part
    for cb in PE_CBS_BY_B[1]:
        emit_conv_pe_cb(1, cb)

    # b1 chains in column halves; C(1) pipelined per half
    acc1 = {}
    for cb in DVE_CBS_BY_B[1]:
        acc1[cb] = emit_chain(1, cb, 0, 1024)
    for cb in DVE_CBS_BY_B[1]:
        emit_chain_prelu(1, cb, 0, 1024, acc1[cb])
    emit_C(1, 0)
    emit_C(1, 1)
    for cb in DVE_CBS_BY_B[1]:
        emit_chain(1, cb, 1024, 1024, acc=acc1[cb])
    for cb in DVE_CBS_BY_B[1]:
        emit_chain_prelu(1, cb, 1024, 1024, acc1[cb])
    emit_C(1, 2)
    emit_C(1, 3)


_NC_CACHE = None


def _get_program():
    global _NC_CACHE
    if _NC_CACHE is None:
        nc = bacc.Bacc("TRN2", target_bir_lowering=False, debug=False)
        with tile.TileContext(nc) as tc, ExitStack() as ctx:
            _build_kernel(ctx, tc)
        nc.compile()
        _NC_CACHE = nc
    return _NC_CACHE


def _host_prep(ln_g, ln_b, w1, b1, dw, dwb, alpha, w2, b2):
    bf = ml_dtypes.bfloat16
    w1 = np.asarray(w1, np.float32)
    ln_g = np.asarray(ln_g, np.float32)
    ln_b = np.asarray(ln_b, np.float32)
    dwf = np.asarray(dw, np.float32).reshape(I, K)
    w1p = np.ascontiguousarray((w1 * ln_g[None, :]).T).astype(bf)
    b1p = (np.asarray(b1, np.float32) + w1 @ ln_b).astype(np.float32)
    w2t = np.ascontiguousarray(np.asarray(w2, np.float32).T).astype(bf)
    diag = np.zeros((P, len(DIAG_CBS) * K * P), np.float32)
    ar = np.arange(P)
    for ci, cb in enumerate(DIAG_CBS):
        for k in range(K):
            diag[ar, (ci * K + k) * P + ar] = dwf[cb * P:(cb + 1) * P, k]
    diag = diag.astype(bf)
    dwc = np.ascontiguousarray(
        dwf.reshape(CB, P, K).transpose(1, 0, 2).reshape(P, CB * K)
    ).astype(np.float32)
    dwbc = np.ascontiguousarray(
        np.asarray(dwb, np.float32).reshape(CB, P).T).astype(np.float32)
    alphac = np.ascontiguousarray(
        np.asarray(alpha, np.float32).reshape(CB, P).T).astype(np.float32)
    b2r = np.asarray(b2, np.float32)[None, :].astype(bf)
    ones = np.ones((1, P), np.float32).astype(bf)
    return {"w1p": w1p, "b1p": b1p, "w2t": w2t, "diag": diag, "dwc": dwc,
            "dwbc": dwbc, "alphac": alphac, "b2r": b2r, "ones": ones}


def kernel(x, ln_g, ln_b, w1, b1, dw, dwb, alpha, w2, b2, _trace=False):
    nc = _get_program()
    x = np.ascontiguousarray(x, dtype=np.float32)
    shared = _host_prep(ln_g, ln_b, w1, b1, dw, dwb, alpha, w2, b2)
    in_maps = [
        {"x": x[c * BPC:(c + 1) * BPC], **shared} for c in range(NCORES)
    ]
    res = run_bass_kernel_spmd(nc, in_maps, core_ids=list(range(NCORES)),
                               trace=_trace)
    out = np.concatenate([res.results[c]["out"] for c in range(NCORES)], axis=0)
    if _trace:
        kernel.last_results = res
    return out
